# revision 45
# baseline (speedup 1.0000x reference)
"""Trainium2 Bass kernel for batched single-head attention with projections.

Reference computation (per batch b):
    Q = q @ Wq + bq ; K = k @ Wk + bk ; V = v @ Wv + bv        (512 -> 64)
    out = softmax(Q K^T / 8) V                                  (S = 4096)

Sharding: 8 cores = 4 batches x 2 kv-sequence halves. Each core gets the
full q for its batch plus its half of k,v (all inputs in fp8-e3m4), all
host-swizzled into the exact [128, chunk, cols] SBUF layout so every DMA
is a flat contiguous per-partition transfer near the SDMA line rate.

Device-side layout trick: everything is computed in "transposed space".
  Q.T [128, 4096] = [Wq|Wq].T @ qT (+bq)   rows 64..127 duplicate 0..63
  K.T [128, 2048] = [Wk|Wk].T @ kT         (bk dropped: softmax-invariant)
  V'  [2048, 65]  = (vT.T @ Wv_aug) + bias ; col 64 == 1.0 (denominator)
  scores.T tile   = K.T-chunk.T @ Q.T-block     -> PSUM [128, 1024]
  P.T             = exp(scores.T / 8)           -> SBUF bf16
  out.T [65, 512] = sum_t V'-tile.T @ P.T-tile  -> PSUM accumulate
Rows 0..63 of out.T are the unnormalized numerator, row 64 the softmax
denominator; the host divides and transposes while unsharding (out is
shipped back in bf16 - the host sums the two kv-half cores in f32).

The scores matmul has contraction dim 64, so pairs of kv-tiles are packed
into the two 64-row halves of the PE array (tile_position row tiling) and
run concurrently. The projections use doubled [W|W] stationaries so one
matmul writes both partition halves (no duplication matmuls). The 64
(scores -> exp -> AV) pair steps are emitted software-pipelined with a
2-step skew so the exp latency never sits on the PE critical path, and
2 of every 8 pairs run their exp on the otherwise-idle DVE via the
Schraudolph bit trick (one tensor_scalar whose int16 output bit pattern
IS bf16(2^x)), splitting the softmax cost across two engines.

Startup choreography (HAM: the PE clock sits at 1.2 GHz until ~3.4us of
continuous activity, then doubles): consts are split into three small
DMAs (wk first) and ka into two halves so the first K-projection can
start ~4us earlier; zero warmup matmuls bridge every DMA wait so the PE
activity window never goes idle before steady state.
"""

import numpy as np
import ml_dtypes

import concourse.bass as bass
import concourse.tile as tile
from concourse import mybir
from concourse.bass_utils import run_bass_kernel_spmd

BF16 = mybir.dt.bfloat16
F32 = mybir.dt.float32
I16 = mybir.dt.int16
FP8 = mybir.dt.float8e3   # e3m4: 4 mantissa bits, max 15.5 — fits randn

# Schraudolph exp-as-int-bits constants: 2^(x*log2e/8) via
# float32_bits = A*x + B; B absorbs the bias-minimizing magic C. The /2^16
# variants produce an int16 whose bit pattern IS bf16(2^(x/8*ln2e)), so a
# single DVE tensor_scalar writes exp() straight into a bf16 tile.
SCH_A = float((1 << 23) * np.log2(np.e) / 8.0)
SCH_B = float(127 * (1 << 23) - 486411)
SCH_A16 = SCH_A / 65536.0
SCH_B16 = SCH_B / 65536.0

B, S, D, E = 4, 4096, 512, 64
H = S                 # q rows per core (full sequence)
KS = S // 2           # kv rows per core (half sequence)
E1 = E + 1            # V' width (ones column appended)
E2 = 2 * E            # doubled projection width ([W|W] stationary)
NCH = D // 128        # contraction chunks (4)
NKV = KS // 128       # kv tiles (16)
NPAIR = NKV // 2      # packed kv tile pairs (8)
QBLK = 512            # sq columns per block
NBLK = H // QBLK      # 8
NSTEP = NBLK * NPAIR  # 64 pipelined pair steps
ESPL = QBLK + 128     # exp-split column: ACT does [0:ESPL], DVE the rest
N_CORES = 8

# (name, n_cols) for the chunked input loads.  The splits are sized so
# each tensor sliver lands just ahead of the pipeline step that consumes
# it (the rings deliver ~1KB/partition/us early on with all 8 cores
# contending): sync carries k + qa + qc, scalar the consts, v and qb.
Q_SPLITS = (("qa", 512), ("qb1", 512), ("qb2", 1024), ("qc", 2048))
K_SPLITS = (("ka0", 256), ("ka1", 256), ("kb", 512), ("kc1", 512),
            ("kc2", 512))
V_SPLITS = (("va", 512), ("vb1", 512), ("vb2", 512), ("vb3", 512))

# const-pack column layouts: cpk = wk; cpq = wq | bq-col; cpv = wv.
# bv is NOT shipped: sum_kv p*(v@Wv + bv) = numerator + bv*denominator,
# so the host just adds bv after the softmax division.
CPK_W = NCH * E2                  # 512
CPQ_W = NCH * E2 + 1              # 513 (bq as a single [128,1] column)
CPV_W = NCH * E                   # 256 (wv chunks, no ones/bias column)


def _build_bass(split_waits: bool = True) -> bass.Bass:
    nc = bass.Bass()
    parms = {}
    for name, w in Q_SPLITS + K_SPLITS + V_SPLITS:
        parms[name] = nc.declare_dram_parameter(name, [128, NCH, w], FP8,
                                                isOutput=False)
    for name, w in (("cpk", CPK_W), ("cpq", CPQ_W), ("cpv", CPV_W)):
        parms[name] = nc.declare_dram_parameter(name, [128, w], BF16,
                                                isOutput=False)
    out = nc.declare_dram_parameter("out", [E1, H], BF16, isOutput=True)

    with tile.TileContext(nc) as tc:
        _body(nc, tc, parms, out)
    if split_waits:  # CoreSim path skips both post-passes (the race
        _gate_dmas(nc)   # detector can't see through the raw sem waits)
        _split_multi_waits(nc)
    return nc


# DMA release epochs (see the comment in _body): each entry gates one DMA
# (and, via engine program order, everything issued after it on the same
# engine) on the completion of earlier-deadline DMAs.
_EPOCH1 = ("qa", "cpq", "cpk", "ka0", "ka1", "cpv")
_DMA_GATES = (
    ("va", _EPOCH1),
    ("kb", _EPOCH1),
    ("vb2", _EPOCH1),
    ("qb2", ("kb", "kc1", "kc2")),
)


def _gate_dmas(nc):
    """Inject standalone semaphore waits in front of the gated DMA
    doorbells (post-scheduling, like _split_multi_waits) so each HWDGE
    queue only becomes active once the previous deadline epoch's
    transfers have fully landed."""
    handles = nc._dma_handles

    # HWDGE queue semaphores are recycled across DMA instructions, so a
    # gate must wait for the CUMULATIVE count on the semaphore up to and
    # including the gating DMA (program order), not a flat +16.
    inst_names = {h.ins.name: n for n, h in handles.items()}
    sem_cum = {}
    wait_tgt = {}
    for blk in nc.m.functions[0].blocks:
        for inst in blk.instructions:
            si = getattr(inst, "sync_info", None)
            if si is None:
                continue
            for u in si.on_update:
                if u.sync_type == "semaphore" and u.update_mode == "sem-add-imm":
                    sem_cum[u.id] = sem_cum.get(u.id, 0) + u.update_value
                    dn = inst_names.get(inst.name)
                    if dn is not None:
                        wait_tgt[dn] = (u.id, sem_cum[u.id])

    def sem_of(name):
        return wait_tgt[name]

    gate_by_target = {handles[g].ins.name: ws for g, ws in _DMA_GATES}
    n = 0
    for blk in nc.m.functions[0].blocks:
        new_insts = []
        for inst in blk.instructions:
            ws = gate_by_target.get(getattr(inst, "name", None))
            if ws is not None:
                for wn in ws:
                    sid, inc = sem_of(wn)
                    n += 1
                    es = mybir.InstEventSemaphore(
                        name=f"DGATE-{n}", ins=[], outs=[])
                    es.engine = inst.engine
                    es.sync_info = mybir.SyncInfo(
                        on_wait=[mybir.SyncWait(
                            sync_type="semaphore", id=sid,
                            ant_name=f"dgate_{wn}",
                            wait_mode="sem-ge-imm", wait_value=inc,
                            wait_reg=None)],
                        on_update=[])
                    new_insts.append(es)
            new_insts.append(inst)
        blk.instructions = new_insts


_NO_SPLIT_OPCODES = {"Drain", "EventSemaphore", "NoOp", "Call", "ISA",
                     "UnconditionalBranch"}


def _split_multi_waits(nc):
    """walrus (this toolchain) encodes at most ONE sem wait per TPB
    instruction (single NEURON_ISA_TPB_EVENTS slot) and refuses to compile
    instructions carrying more. Tile emits multi-wait sync_info freely, so
    split: keep the first wait on the instruction, hoist the rest onto
    standalone EventSemaphore waits just before it on the same engine."""
    n = 0
    for blk in nc.m.functions[0].blocks:
        new_insts = []
        for inst in blk.instructions:
            si = inst.sync_info
            if (si is not None and si.on_wait and len(si.on_wait) > 1
                    and inst.concise_opcode not in _NO_SPLIT_OPCODES):
                waits = list(si.on_wait)
                for w in waits[:-1]:
                    n += 1
                    es = mybir.InstEventSemaphore(
                        name=f"WSPLIT-{n}", ins=[], outs=[])
                    es.engine = inst.engine
                    es.sync_info = mybir.SyncInfo(on_wait=[w], on_update=[])
                    new_insts.append(es)
                inst.sync_info = mybir.SyncInfo(
                    on_wait=[waits[-1]], on_update=list(si.on_update))
            new_insts.append(inst)
        blk.instructions = new_insts


def _body(nc, tc, parms, out):
    with (
        tc.tile_pool(name="consts", bufs=1) as cst,
        tc.tile_pool(name="raw", bufs=1) as raw,
        tc.tile_pool(name="proj", bufs=1) as proj,
        tc.tile_pool(name="pt", bufs=16) as ptp,
        tc.tile_pool(name="ob", bufs=4) as obp,
        tc.tile_pool(name="ps", bufs=2, space="PSUM") as ps,
        tc.tile_pool(name="psc", bufs=2, space="PSUM") as psc,
        tc.tile_pool(name="pso", bufs=2, space="PSUM") as pso,
    ):
        # HAM warmup tile: zero matmuls keep the PE activity monitor busy
        # from the runtime preamble until real data lands, so the clock
        # gate opens (1.2 -> 2.4 GHz) before the first projections.
        wz = cst.tile([128, 512], BF16, tag="wz")
        nc.gpsimd.memset(wz, 0)
        warm = ps.tile([128, 512], F32, tag="ps_main")

        def warmup(n):
            for w in range(n):
                nc.tensor.matmul(warm[:, :], wz[:, 0:128], wz[:, :],
                                 start=(w == 0), stop=(w == n - 1))

        # DMA epochs: the HWDGE queues of a ring progress ROUND-ROBIN, so
        # issue order alone gives no priority — a big late-deadline load
        # steals bandwidth from the prologue-critical ones.  The loads are
        # therefore released in deadline epochs, enforced by semaphore
        # gates injected post-scheduling (_gate_dmas):
        #   epoch 1 (prologue):  qa cpq cpk | ka0 ka1 cpv   (~2-3 queues
        #       per ring: a lone queue only reaches ~half the ring rate)
        #   epoch 2 (block 0):   va vb1 qb1 | kb kc1 kc2
        #   epoch 3:             qb2 qc    (sync, after epoch 2's k side)
        #   gpsimd SW-DGE, ungated (it is fast and done by ~13us): vb2 vb3
        dma_h = {}
        cpq = cst.tile([128, CPQ_W], BF16, tag="cpq")
        dma_h["cpq"] = nc.scalar.dma_start(out=cpq, in_=parms["cpq"][:, :])
        cpk = cst.tile([128, CPK_W], BF16, tag="cpk")
        dma_h["cpk"] = nc.scalar.dma_start(out=cpk, in_=parms["cpk"][:, :])
        cpv = cst.tile([128, CPV_W], BF16, tag="cpv")
        dma_h["cpv"] = nc.sync.dma_start(out=cpv, in_=parms["cpv"][:, :])
        wk_sb = cpk[:, :]
        wq_sb = cpq[:, 0:NCH * E2]
        bq_col = cpq[:, NCH * E2:NCH * E2 + 1]
        wv_sb = cpv[:, :]
        # preload the exp table set off the critical path (first real exp
        # otherwise eats the ~2.7us ACT_TABLE_LOAD mid-pipeline); hangs off
        # the first const sliver so the table DMA fires immediately
        scr = cst.tile([1, 8], F32, tag="scr")
        nc.scalar.activation(scr[:, :], cpq[0:1, 0:8],
                             mybir.ActivationFunctionType.Exp)
        # tensor_scalar wants its per-partition scalar operand in f32
        bqf = cst.tile([128, 1], F32, tag="bqf")
        nc.vector.tensor_copy(bqf, bq_col)

        # raw inputs, host-swizzled to the SBUF layout: each DMA is a
        # flat [128, chunk*cols] contiguous-per-partition transfer.
        tiles = {}

        def load(eng, name):
            t = raw.tile(list(parms[name].shape), FP8, tag=name)
            dma_h[name] = eng.dma_start(out=t, in_=parms[name][:, :, :])
            tiles[name] = t

        load(nc.scalar, "qa")
        load(nc.sync, "ka0")
        load(nc.sync, "ka1")
        load(nc.scalar, "va")
        load(nc.scalar, "vb1")
        load(nc.scalar, "qb1")
        load(nc.sync, "kb")
        load(nc.sync, "kc1")
        load(nc.sync, "kc2")
        load(nc.sync, "qb2")
        load(nc.sync, "qc")
        load(nc.gpsimd, "vb2")
        load(nc.gpsimd, "vb3")
        nc._dma_handles = dma_h

        def qt_slice(c, blk):
            if blk == 0:
                return tiles["qa"][:, c, :]
            if blk == 1:
                return tiles["qb1"][:, c, :]
            if blk < 4:
                return tiles["qb2"][:, c, (blk - 2) * 512:(blk - 1) * 512]
            return tiles["qc"][:, c, (blk - 4) * 512:(blk - 3) * 512]

        def kt_slice(c, blk):       # 512-col K projection block (1..3)
            return tiles[("kb", "kc1", "kc2")[blk - 1]][:, c, :]

        def vt_slice(c, t):         # 128-col V tile
            name = ("va", "vb1", "vb2", "vb3")[t // 4]
            return tiles[name][:, c, (t % 4) * 128:(t % 4 + 1) * 128]

        # projected tensors; Q.T/K.T have rows 0..63 duplicated into
        # 64..127 (written in one pass via the doubled stationaries) so
        # the scores matmuls can row-pack both PE array halves
        QT2 = proj.tile([128, H], BF16, tag="QT2")
        KT2 = proj.tile([128, KS], BF16, tag="KT2")
        Vp = proj.tile([128, NKV, E1], BF16, tag="Vp")
        # column 64 of every V' tile is the softmax-denominator ones
        # column — written once here, never touched by the projections
        nc.gpsimd.memset(Vp[:, :, E:E1], 1.0)

        def q_proj(blk):
            acc = ps.tile([128, 512], F32, tag="ps_main")
            sl = slice(blk * 512, (blk + 1) * 512)
            for c in range(NCH):
                nc.tensor.matmul(
                    acc[:, :], wq_sb[:, c * E2:(c + 1) * E2], qt_slice(c, blk),
                    start=(c == 0), stop=(c == NCH - 1),
                )
            nc.vector.tensor_scalar(
                QT2[:, sl], acc[:, :], bqf, None,
                op0=mybir.AluOpType.add,
            )

        def k_proj_half(name, sl):  # 256-col half of K block 0
            acc = ps.tile([128, 512], F32, tag="ps_main")
            for c in range(NCH):
                nc.tensor.matmul(
                    acc[:, 0:256], wk_sb[:, c * E2:(c + 1) * E2],
                    tiles[name][:, c, :],
                    start=(c == 0), stop=(c == NCH - 1),
                )
            nc.vector.tensor_copy(KT2[:, sl], acc[:, 0:256])

        def k_proj(blk):
            acc = ps.tile([128, 512], F32, tag="ps_main")
            sl = slice(blk * 512, (blk + 1) * 512)
            for c in range(NCH):
                nc.tensor.matmul(
                    acc[:, :], wk_sb[:, c * E2:(c + 1) * E2], kt_slice(c, blk),
                    start=(c == 0), stop=(c == NCH - 1),
                )
            nc.vector.tensor_copy(KT2[:, sl], acc[:, :])

        def v_proj4(g):
            # four V tiles batched through one PSUM accumulator: one
            # strided copy epilogue instead of four bias-adds, so the
            # PSUM pool wraps (and couples the PE to the DVE) 4x less
            acc = ps.tile([128, 4, E], F32, tag="ps_main")
            for j in range(4):
                t = 4 * g + j
                for c in range(NCH):
                    nc.tensor.matmul(
                        acc[:, j, :], vt_slice(c, t),
                        wv_sb[:, c * E:(c + 1) * E],
                        start=(c == 0), stop=(c == NCH - 1),
                    )
            nc.vector.tensor_copy(Vp[:, 4 * g:4 * g + 4, 0:E], acc[:, :, :])

        # ---- software-pipelined attention: 64 pair steps, skew 2 ----
        sc_t = [None] * NSTEP
        pt_t = [None] * NSTEP
        acc = {}

        def scores(i):
            blk, p = divmod(i, NPAIR)
            sq = slice(blk * QBLK, (blk + 1) * QBLK)
            sc = psc.tile([128, 2 * QBLK], F32, tag="ps_sc")
            nc.tensor.matmul(
                sc[:, 0:QBLK],
                KT2[0:E, (2 * p) * 128:(2 * p + 1) * 128],
                QT2[0:E, sq],
                start=True, stop=True, tile_position=(0, 0),
            )
            nc.tensor.matmul(
                sc[:, QBLK:2 * QBLK],
                KT2[E:2 * E, (2 * p + 1) * 128:(2 * p + 2) * 128],
                QT2[E:2 * E, sq],
                start=True, stop=True, tile_position=(64, 0),
            )
            sc_t[i] = sc

        # The exp of each pair step is split across both engines: ACT
        # exponentiates the first ESPL columns, the DVE does the rest via
        # the Schraudolph bit trick (int16(round(A*x + B)) IS the bf16
        # bit pattern of 2^(cx), so one tensor_scalar writes exp() into
        # the bf16 pt tile directly).  The split point balances the two
        # engines (ACT ~1.09 ns/col vs DVE ~1.33 ns/col + its epilogue/
        # output-cast duties), so neither engine's exp chain paces the PE
        # (the psc pool only holds 2 score tiles, so scores(i+2) waits
        # for exp(i) to drain).  Block 0 runs full-ACT exps: its steps
        # are filler-stretched anyway and the DVE must stay responsive
        # for the projection epilogues.
        def expq(i):
            pt = ptp.tile([128, 2 * QBLK], BF16, tag="pt")
            if i < 8:
                nc.scalar.activation(
                    pt[:, :], sc_t[i][:, :],
                    mybir.ActivationFunctionType.Exp, scale=0.125,
                )
            else:
                nc.scalar.activation(
                    pt[:, 0:ESPL], sc_t[i][:, 0:ESPL],
                    mybir.ActivationFunctionType.Exp, scale=0.125,
                )
                nc.vector.tensor_scalar(
                    pt[:, ESPL:2 * QBLK].bitcast(I16),
                    sc_t[i][:, ESPL:2 * QBLK], SCH_A16, SCH_B16,
                    op0=mybir.AluOpType.mult, op1=mybir.AluOpType.add,
                )
            pt_t[i] = pt

        def av(i):
            blk, p = divmod(i, NPAIR)
            if p == 0:
                acc[blk] = pso.tile([E1, QBLK], F32, tag="ps_out",
                                    name=f"acc{blk}")
            a = acc[blk]
            nc.tensor.matmul(
                a[:, :], Vp[:, 2 * p, :], pt_t[i][:, 0:QBLK],
                start=(p == 0), stop=False,
            )
            nc.tensor.matmul(
                a[:, :], Vp[:, 2 * p + 1, :], pt_t[i][:, QBLK:2 * QBLK],
                start=False, stop=(p == NPAIR - 1),
            )
            if p == NPAIR - 1:
                # two half-epilogues (bf16) on both HWDGE rings shorten the
                # copy->dma tail after the final pair; the last block runs
                # its two casts on DVE + gpsimd in parallel
                c0 = blk * QBLK
                ob = obp.tile([E1, QBLK], BF16, tag="ob")
                nc.vector.tensor_copy(ob[:, 0:256], a[:, 0:256])
                nc.sync.dma_start(out=out[:, c0:c0 + 256], in_=ob[:, 0:256])
                nc.vector.tensor_copy(ob[:, 256:512], a[:, 256:512])
                # mid-run out-DMAs stay off the scalar engine (its doorbell
                # would steal ~0.7us from the exp stream); the final block
                # splits across both rings to shorten the drain tail
                oeng = nc.scalar if blk == NBLK - 1 else nc.sync
                oeng.dma_start(out=out[:, c0 + 256:c0 + 512],
                               in_=ob[:, 256:512])

        # filler (projection) work attached ahead of specific steps so it
        # lands in the PE's exp-wait slack and tracks DMA arrival order.
        # Constraints: scores(i) consumes k_proj of its KT block at the
        # TOP of step i, av(i-2) consumes Vp tiles before the fillers run,
        # and every tensor's DMA must have landed (deadline-ordered rings).
        fillers = {
            0: [lambda: k_proj_half("ka1", slice(256, 512))],
            1: [lambda: k_proj(1), lambda: v_proj4(0)],
            3: [lambda: k_proj(2), lambda: v_proj4(1)],
            5: [lambda: k_proj(3), lambda: v_proj4(2)],
            7: [lambda: q_proj(1), lambda: v_proj4(3)],
        }
        for b in range(2, NBLK):
            fillers.setdefault(8 * (b - 1) + 4, []).append(
                lambda b=b: q_proj(b))

        # prologue: warmup matmuls bridge the DMA wait so the PE activity
        # window stays continuously busy from ~8.3us until steady state
        warmup(7)
        q_proj(0)
        k_proj_half("ka0", slice(0, 256))

        # skew-2 pipeline: scores run two pairs ahead of the AV consumer so
        # the exp latency (ACT or DVE) never sits on the PE critical path.
        # expq is emitted LAST in each step: the DVE/ACT completion sems
        # are monotonic per engine, so any projection epilogue queued
        # after an exp would drag the exp's ~1.2us into every PE matmul
        # that waits on that epilogue (PSUM-pool reuse).
        scores(0)
        for f in fillers.get(0, []):
            f()
        expq(0)
        scores(1)
        for f in fillers.get(1, []):
            f()
        expq(1)
        for i in range(2, NSTEP):
            scores(i)
            # AV of the pair whose exp completed a step ago goes ahead of
            # the projection fillers, so it is never queued behind ~1us of
            # proj matmuls it does not depend on
            av(i - 2)
            for f in fillers.get(i, []):
                f()
            expq(i)
        av(NSTEP - 2)
        av(NSTEP - 1)


_CACHED_NC = None


def _get_nc():
    global _CACHED_NC
    if _CACHED_NC is None:
        _CACHED_NC = _build_bass()
    return _CACHED_NC


def _swizzle_w(w: np.ndarray, double: bool = False) -> np.ndarray:
    """[512, width] -> [128, NCH*width] with chunk-major free dim.
    double=True emits [W|W] chunks ([128, NCH*2*width]) so one matmul
    writes the projection into both partition halves."""
    width = w.shape[1]
    c = w.reshape(NCH, 128, width)
    if double:
        c = np.concatenate([c, c], axis=2)
        width *= 2
    return np.ascontiguousarray(
        c.transpose(1, 0, 2).reshape(128, NCH * width)
    ).astype(ml_dtypes.bfloat16)


def _chunk3d(xT: np.ndarray, splits, dt) -> dict:
    """[512, N] (transposed input) -> per-split [128, NCH, w] arrays in
    the exact SBUF tile layout (partition p, chunk c) = row c*128+p."""
    x = np.asarray(xT, np.float32).reshape(NCH, 128, xT.shape[1])
    outmaps = {}
    c0 = 0
    for name, w in splits:
        outmaps[name] = np.ascontiguousarray(
            x[:, :, c0:c0 + w].transpose(1, 0, 2)).astype(dt)
        c0 += w
    return outmaps


def _make_in_maps(q, k, v, Wq, bq, Wk, bk, Wv, bv):
    del bk  # constant along the kv axis -> softmax-invariant, dropped
    del bv  # applied on the host after the softmax division
    bf = ml_dtypes.bfloat16
    f8 = ml_dtypes.float8_e3m4
    wq_s = _swizzle_w(np.asarray(Wq, np.float32), double=True)
    wk_s = _swizzle_w(np.asarray(Wk, np.float32), double=True)
    wv_s = _swizzle_w(np.asarray(Wv, np.float32))
    bq_col = np.asarray(bq, np.float32).reshape(E, 1)
    bq_a = np.ascontiguousarray(
        np.concatenate([bq_col, bq_col], axis=0)).astype(bf)  # [128, 1]
    cpk_a = wk_s
    cpq_a = np.ascontiguousarray(np.concatenate([wq_s, bq_a], axis=1))
    cpv_a = wv_s
    assert cpk_a.shape == (128, CPK_W)
    assert cpq_a.shape == (128, CPQ_W)
    assert cpv_a.shape == (128, CPV_W)

    in_maps = []
    for core in range(N_CORES):
        b, h = core // 2, core % 2
        m = {"cpk": cpk_a, "cpq": cpq_a, "cpv": cpv_a}
        m.update(_chunk3d(np.asarray(q[b], np.float32).T, Q_SPLITS, f8))
        m.update(_chunk3d(
            np.asarray(k[b, h * KS:(h + 1) * KS, :], np.float32).T,
            K_SPLITS, f8))
        m.update(_chunk3d(
            np.asarray(v[b, h * KS:(h + 1) * KS, :], np.float32).T,
            V_SPLITS, f8))
        in_maps.append(m)
    return in_maps


def _unshard(results, bv) -> np.ndarray:
    bv_row = np.asarray(bv, np.float32)
    final = np.empty((B, S, E), np.float32)
    for b in range(B):
        o = (np.asarray(results[2 * b]["out"], np.float32)
             + np.asarray(results[2 * b + 1]["out"], np.float32))  # [65, S]
        final[b] = (o[:E] / o[E:E + 1]).T + bv_row
    return final


def kernel(q, k, v, Wq, bq, Wk, bk, Wv, bv, _trace=False):
    nc = _get_nc()
    in_maps = _make_in_maps(q, k, v, Wq, bq, Wk, bk, Wv, bv)
    res = run_bass_kernel_spmd(nc, in_maps, core_ids=list(range(N_CORES)),
                               trace=_trace)
    outp = _unshard(res.results, bv)
    if _trace:
        kernel.last_result = res
    return outp


# revision 48
# speedup vs baseline: 1.0427x; 1.0427x over previous
"""Trainium2 Bass kernel for batched single-head attention with projections.

Reference computation (per batch b):
    Q = q @ Wq + bq ; K = k @ Wk + bk ; V = v @ Wv + bv        (512 -> 64)
    out = softmax(Q K^T / 8) V                                  (S = 4096)

Sharding: 8 cores = 4 batches x 2 kv-sequence halves. Each core gets the
full q for its batch plus its half of k,v (all inputs in fp8-e3m4), all
host-swizzled into the exact [128, chunk, cols] SBUF layout so every DMA
is a flat contiguous per-partition transfer near the SDMA line rate.

Device-side layout trick: everything is computed in "transposed space".
  Q.T [128, 4096] = [Wq|Wq].T @ qT (+bq)   rows 64..127 duplicate 0..63
  K.T [128, 2048] = [Wk|Wk].T @ kT         (bk dropped: softmax-invariant)
  V'  [2048, 65]  = (vT.T @ Wv_aug) + bias ; col 64 == 1.0 (denominator)
  scores.T tile   = K.T-chunk.T @ Q.T-block     -> PSUM [128, 1024]
  P.T             = exp(scores.T / 8)           -> SBUF bf16
  out.T [65, 512] = sum_t V'-tile.T @ P.T-tile  -> PSUM accumulate
Rows 0..63 of out.T are the unnormalized numerator, row 64 the softmax
denominator; the host divides and transposes while unsharding (out is
shipped back in bf16 - the host sums the two kv-half cores in f32).

The scores matmul has contraction dim 64, so pairs of kv-tiles are packed
into the two 64-row halves of the PE array (tile_position row tiling) and
run concurrently. The projections use doubled [W|W] stationaries so one
matmul writes both partition halves (no duplication matmuls). The 64
(scores -> exp -> AV) pair steps are emitted software-pipelined with a
2-step skew so the exp latency never sits on the PE critical path, and
2 of every 8 pairs run their exp on the otherwise-idle DVE via the
Schraudolph bit trick (one tensor_scalar whose int16 output bit pattern
IS bf16(2^x)), splitting the softmax cost across two engines.

Startup choreography (HAM: the PE clock sits at 1.2 GHz until ~3.4us of
continuous activity, then doubles): consts are split into three small
DMAs (wk first) and ka into two halves so the first K-projection can
start ~4us earlier; zero warmup matmuls bridge every DMA wait so the PE
activity window never goes idle before steady state.
"""

import numpy as np
import ml_dtypes

import concourse.bass as bass
import concourse.tile as tile
from concourse import mybir
from concourse.bass_utils import run_bass_kernel_spmd

BF16 = mybir.dt.bfloat16
F32 = mybir.dt.float32
I16 = mybir.dt.int16
FP8 = mybir.dt.float8e3   # e3m4: 4 mantissa bits, max 15.5 — fits randn

# Schraudolph exp-as-int-bits constants: 2^(x*log2e/8) via
# float32_bits = A*x + B; B absorbs the bias-minimizing magic C. The /2^16
# variants produce an int16 whose bit pattern IS bf16(2^(x/8*ln2e)), so a
# single DVE tensor_scalar writes exp() straight into a bf16 tile.
SCH_A = float((1 << 23) * np.log2(np.e) / 8.0)
SCH_B = float(127 * (1 << 23) - 486411)
SCH_A16 = SCH_A / 65536.0
SCH_B16 = SCH_B / 65536.0

B, S, D, E = 4, 4096, 512, 64
H = S                 # q rows per core (full sequence)
KS = S // 2           # kv rows per core (half sequence)
E1 = E + 1            # V' width (ones column appended)
E2 = 2 * E            # doubled projection width ([W|W] stationary)
NCH = D // 128        # contraction chunks (4)
NKV = KS // 128       # kv tiles (16)
NPAIR = NKV // 2      # packed kv tile pairs (8)
QBLK = 512            # sq columns per block
NBLK = H // QBLK      # 8
NSTEP = NBLK * NPAIR  # 64 pipelined pair steps
ESPL = QBLK + 128     # exp-split column: ACT does [0:ESPL], DVE the rest
N_CORES = 8

# (name, n_cols) for the chunked input loads.  The splits are sized so
# each tensor sliver lands just ahead of the pipeline step that consumes
# it (the rings deliver ~1KB/partition/us early on with all 8 cores
# contending): sync carries k + qa + qc, scalar the consts, v and qb.
Q_SPLITS = (("qa", 512), ("qb1", 512), ("qb2", 1024), ("qc", 2048))
K_SPLITS = (("ka0", 256), ("ka1", 256), ("kb", 512), ("kc1", 512),
            ("kc2", 512))
V_SPLITS = (("va", 512), ("vb1", 512), ("vb2", 512), ("vb3", 512))

# const-pack column layouts: cpk = wk; cpq = wq | bq-col; cpv = wv.
# bv is NOT shipped: sum_kv p*(v@Wv + bv) = numerator + bv*denominator,
# so the host just adds bv after the softmax division.
CPK_W = NCH * E2                  # 512
CPQ_W = NCH * E2 + 1              # 513 (bq as a single [128,1] column)
CPV_W = NCH * E                   # 256 (wv chunks, no ones/bias column)


def _build_bass(split_waits: bool = True) -> bass.Bass:
    nc = bass.Bass()
    parms = {}
    for name, w in Q_SPLITS + K_SPLITS + V_SPLITS:
        parms[name] = nc.declare_dram_parameter(name, [128, NCH, w], FP8,
                                                isOutput=False)
    for name, w in (("cpk", CPK_W), ("cpq", CPQ_W), ("cpv", CPV_W)):
        parms[name] = nc.declare_dram_parameter(name, [128, w], BF16,
                                                isOutput=False)
    out = nc.declare_dram_parameter("out", [E1, H], BF16, isOutput=True)

    with tile.TileContext(nc) as tc:
        _body(nc, tc, parms, out)
    if split_waits:  # CoreSim path skips both post-passes (the race
        _gate_dmas(nc)   # detector can't see through the raw sem waits)
        _split_multi_waits(nc)
    return nc


# DMA release epochs (see the comment in _body): each entry gates one DMA
# (and, via engine program order, everything issued after it on the same
# engine) on the completion of earlier-deadline DMAs.
_EPOCH1 = ("qa", "cpq", "cpk", "ka0", "ka1", "cpv")
_DMA_GATES = (
    ("va", _EPOCH1),
    ("kb", _EPOCH1),
    ("vb2", _EPOCH1),
    ("qb2", ("kb", "kc1", "kc2")),
)


def _gate_dmas(nc):
    """Inject standalone semaphore waits in front of the gated DMA
    doorbells (post-scheduling, like _split_multi_waits) so each HWDGE
    queue only becomes active once the previous deadline epoch's
    transfers have fully landed."""
    handles = nc._dma_handles

    # HWDGE queue semaphores are recycled across DMA instructions, so a
    # gate must wait for the CUMULATIVE count on the semaphore up to and
    # including the gating DMA (program order), not a flat +16.
    inst_names = {h.ins.name: n for n, h in handles.items()}
    sem_cum = {}
    wait_tgt = {}
    for blk in nc.m.functions[0].blocks:
        for inst in blk.instructions:
            si = getattr(inst, "sync_info", None)
            if si is None:
                continue
            for u in si.on_update:
                if u.sync_type == "semaphore" and u.update_mode == "sem-add-imm":
                    sem_cum[u.id] = sem_cum.get(u.id, 0) + u.update_value
                    dn = inst_names.get(inst.name)
                    if dn is not None:
                        wait_tgt[dn] = (u.id, sem_cum[u.id])

    def sem_of(name):
        return wait_tgt[name]

    gate_by_target = {handles[g].ins.name: ws for g, ws in _DMA_GATES}
    n = 0
    for blk in nc.m.functions[0].blocks:
        new_insts = []
        for inst in blk.instructions:
            ws = gate_by_target.get(getattr(inst, "name", None))
            if ws is not None:
                for wn in ws:
                    sid, inc = sem_of(wn)
                    n += 1
                    es = mybir.InstEventSemaphore(
                        name=f"DGATE-{n}", ins=[], outs=[])
                    es.engine = inst.engine
                    es.sync_info = mybir.SyncInfo(
                        on_wait=[mybir.SyncWait(
                            sync_type="semaphore", id=sid,
                            ant_name=f"dgate_{wn}",
                            wait_mode="sem-ge-imm", wait_value=inc,
                            wait_reg=None)],
                        on_update=[])
                    new_insts.append(es)
            new_insts.append(inst)
        blk.instructions = new_insts


_NO_SPLIT_OPCODES = {"Drain", "EventSemaphore", "NoOp", "Call", "ISA",
                     "UnconditionalBranch"}


def _split_multi_waits(nc):
    """walrus (this toolchain) encodes at most ONE sem wait per TPB
    instruction (single NEURON_ISA_TPB_EVENTS slot) and refuses to compile
    instructions carrying more. Tile emits multi-wait sync_info freely, so
    split: keep the first wait on the instruction, hoist the rest onto
    standalone EventSemaphore waits just before it on the same engine."""
    n = 0
    for blk in nc.m.functions[0].blocks:
        new_insts = []
        for inst in blk.instructions:
            si = inst.sync_info
            if (si is not None and si.on_wait and len(si.on_wait) > 1
                    and inst.concise_opcode not in _NO_SPLIT_OPCODES):
                waits = list(si.on_wait)
                for w in waits[:-1]:
                    n += 1
                    es = mybir.InstEventSemaphore(
                        name=f"WSPLIT-{n}", ins=[], outs=[])
                    es.engine = inst.engine
                    es.sync_info = mybir.SyncInfo(on_wait=[w], on_update=[])
                    new_insts.append(es)
                inst.sync_info = mybir.SyncInfo(
                    on_wait=[waits[-1]], on_update=list(si.on_update))
            new_insts.append(inst)
        blk.instructions = new_insts


def _body(nc, tc, parms, out):
    with (
        tc.tile_pool(name="consts", bufs=1) as cst,
        tc.tile_pool(name="raw", bufs=1) as raw,
        tc.tile_pool(name="proj", bufs=1) as proj,
        tc.tile_pool(name="pt", bufs=16) as ptp,
        tc.tile_pool(name="ob", bufs=4) as obp,
        tc.tile_pool(name="ps", bufs=2, space="PSUM") as ps,
        tc.tile_pool(name="psc", bufs=4, space="PSUM") as psc,
        tc.tile_pool(name="pso", bufs=2, space="PSUM") as pso,
    ):
        # HAM warmup tile: zero matmuls keep the PE activity monitor busy
        # from the runtime preamble until real data lands, so the clock
        # gate opens (1.2 -> 2.4 GHz) before the first projections.
        wz = cst.tile([128, 512], BF16, tag="wz")
        nc.gpsimd.memset(wz, 0)
        warm = ps.tile([128, 512], F32, tag="ps_main")

        def warmup(n):
            for w in range(n):
                nc.tensor.matmul(warm[:, :], wz[:, 0:128], wz[:, :],
                                 start=(w == 0), stop=(w == n - 1))

        # DMA epochs: the HWDGE queues of a ring progress ROUND-ROBIN, so
        # issue order alone gives no priority — a big late-deadline load
        # steals bandwidth from the prologue-critical ones.  The loads are
        # therefore released in deadline epochs, enforced by semaphore
        # gates injected post-scheduling (_gate_dmas):
        #   epoch 1 (prologue):  qa cpq cpk | ka0 ka1 cpv   (~2-3 queues
        #       per ring: a lone queue only reaches ~half the ring rate)
        #   epoch 2 (block 0):   va vb1 qb1 | kb kc1 kc2
        #   epoch 3:             qb2 qc    (sync, after epoch 2's k side)
        #   gpsimd SW-DGE, ungated (it is fast and done by ~13us): vb2 vb3
        dma_h = {}
        cpq = cst.tile([128, CPQ_W], BF16, tag="cpq")
        dma_h["cpq"] = nc.scalar.dma_start(out=cpq, in_=parms["cpq"][:, :])
        cpk = cst.tile([128, CPK_W], BF16, tag="cpk")
        dma_h["cpk"] = nc.scalar.dma_start(out=cpk, in_=parms["cpk"][:, :])
        cpv = cst.tile([128, CPV_W], BF16, tag="cpv")
        dma_h["cpv"] = nc.sync.dma_start(out=cpv, in_=parms["cpv"][:, :])
        wk_sb = cpk[:, :]
        wq_sb = cpq[:, 0:NCH * E2]
        bq_col = cpq[:, NCH * E2:NCH * E2 + 1]
        wv_sb = cpv[:, :]
        # preload the exp table set off the critical path (first real exp
        # otherwise eats the ~2.7us ACT_TABLE_LOAD mid-pipeline); hangs off
        # the first const sliver so the table DMA fires immediately
        scr = cst.tile([1, 8], F32, tag="scr")
        nc.scalar.activation(scr[:, :], cpq[0:1, 0:8],
                             mybir.ActivationFunctionType.Exp)
        # tensor_scalar wants its per-partition scalar operand in f32
        bqf = cst.tile([128, 1], F32, tag="bqf")
        nc.vector.tensor_copy(bqf, bq_col)

        # raw inputs, host-swizzled to the SBUF layout: each DMA is a
        # flat [128, chunk*cols] contiguous-per-partition transfer.
        tiles = {}

        def load(eng, name):
            t = raw.tile(list(parms[name].shape), FP8, tag=name)
            dma_h[name] = eng.dma_start(out=t, in_=parms[name][:, :, :])
            tiles[name] = t

        load(nc.scalar, "qa")
        load(nc.sync, "ka0")
        load(nc.sync, "ka1")
        load(nc.scalar, "va")
        load(nc.scalar, "vb1")
        load(nc.scalar, "qb1")
        load(nc.sync, "kb")
        load(nc.sync, "kc1")
        load(nc.sync, "kc2")
        load(nc.sync, "qb2")
        load(nc.sync, "qc")
        load(nc.gpsimd, "vb2")
        load(nc.gpsimd, "vb3")
        nc._dma_handles = dma_h

        def qt_slice(c, blk):
            if blk == 0:
                return tiles["qa"][:, c, :]
            if blk == 1:
                return tiles["qb1"][:, c, :]
            if blk < 4:
                return tiles["qb2"][:, c, (blk - 2) * 512:(blk - 1) * 512]
            return tiles["qc"][:, c, (blk - 4) * 512:(blk - 3) * 512]

        def kt_slice(c, blk):       # 512-col K projection block (1..3)
            return tiles[("kb", "kc1", "kc2")[blk - 1]][:, c, :]

        def vt_slice(c, t):         # 128-col V tile
            name = ("va", "vb1", "vb2", "vb3")[t // 4]
            return tiles[name][:, c, (t % 4) * 128:(t % 4 + 1) * 128]

        # projected tensors; Q.T/K.T have rows 0..63 duplicated into
        # 64..127 (written in one pass via the doubled stationaries) so
        # the scores matmuls can row-pack both PE array halves
        QT2 = proj.tile([128, H], BF16, tag="QT2")
        KT2 = proj.tile([128, KS], BF16, tag="KT2")
        Vp = proj.tile([128, NKV, E1], BF16, tag="Vp")
        # column 64 of every V' tile is the softmax-denominator ones
        # column — written once here, never touched by the projections
        nc.gpsimd.memset(Vp[:, :, E:E1], 1.0)

        def q_proj(blk):
            acc = ps.tile([128, 512], F32, tag="ps_main")
            sl = slice(blk * 512, (blk + 1) * 512)
            for c in range(NCH):
                nc.tensor.matmul(
                    acc[:, :], wq_sb[:, c * E2:(c + 1) * E2], qt_slice(c, blk),
                    start=(c == 0), stop=(c == NCH - 1),
                )
            nc.vector.tensor_scalar(
                QT2[:, sl], acc[:, :], bqf, None,
                op0=mybir.AluOpType.add,
            )

        def k_proj_half(name, sl):  # 256-col half of K block 0
            acc = ps.tile([128, 512], F32, tag="ps_main")
            for c in range(NCH):
                nc.tensor.matmul(
                    acc[:, 0:256], wk_sb[:, c * E2:(c + 1) * E2],
                    tiles[name][:, c, :],
                    start=(c == 0), stop=(c == NCH - 1),
                )
            nc.vector.tensor_copy(KT2[:, sl], acc[:, 0:256])

        def k_proj(blk):
            acc = ps.tile([128, 512], F32, tag="ps_main")
            sl = slice(blk * 512, (blk + 1) * 512)
            for c in range(NCH):
                nc.tensor.matmul(
                    acc[:, :], wk_sb[:, c * E2:(c + 1) * E2], kt_slice(c, blk),
                    start=(c == 0), stop=(c == NCH - 1),
                )
            nc.vector.tensor_copy(KT2[:, sl], acc[:, :])

        def v_proj4(g):
            # four V tiles batched through one PSUM accumulator: one
            # strided copy epilogue instead of four bias-adds, so the
            # PSUM pool wraps (and couples the PE to the DVE) 4x less
            acc = ps.tile([128, 4, E], F32, tag="ps_main")
            for j in range(4):
                t = 4 * g + j
                for c in range(NCH):
                    nc.tensor.matmul(
                        acc[:, j, :], vt_slice(c, t),
                        wv_sb[:, c * E:(c + 1) * E],
                        start=(c == 0), stop=(c == NCH - 1),
                    )
            nc.vector.tensor_copy(Vp[:, 4 * g:4 * g + 4, 0:E], acc[:, :, :])

        # ---- software-pipelined attention: 64 pair steps, skew 2 ----
        sc_t = [None] * NSTEP
        pt_t = [None] * NSTEP
        acc = {}

        def scores(i):
            # the pair's two halves go to SEPARATE single-bank PSUM tiles:
            # the h0/ACT and h1/DVE exp chains then recycle score buffers
            # independently, so neither engine's WAR loop drags the other
            blk, p = divmod(i, NPAIR)
            sq = slice(blk * QBLK, (blk + 1) * QBLK)
            sca = psc.tile([128, QBLK], F32, tag="ps_sc")
            scb = psc.tile([128, QBLK], F32, tag="ps_sc")
            nc.tensor.matmul(
                sca[:, :],
                KT2[0:E, (2 * p) * 128:(2 * p + 1) * 128],
                QT2[0:E, sq],
                start=True, stop=True, tile_position=(0, 0),
            )
            nc.tensor.matmul(
                scb[:, :],
                KT2[E:2 * E, (2 * p + 1) * 128:(2 * p + 2) * 128],
                QT2[E:2 * E, sq],
                start=True, stop=True, tile_position=(64, 0),
            )
            sc_t[i] = (sca, scb)

        # The exp of each pair step is split across both engines: ACT
        # exponentiates the even kv tile (h0) while the DVE does the odd
        # one (h1) via the Schraudolph bit trick (int16(round(A*x + B))
        # IS the bf16 bit pattern of 2^(cx), so one tensor_scalar writes
        # exp() into the bf16 pt tile directly).  ~650ns on each engine
        # in parallel instead of ~1.1us serialized on one, so the
        # score-buffer WAR loop (scores(i+2) waits exp(i)) never paces
        # the PE.  Block 0 runs full-ACT exps: its steps are filler-
        # stretched anyway and the DVE must stay responsive for the
        # projection epilogues.
        def expq(i):
            pt = ptp.tile([128, 2 * QBLK], BF16, tag="pt")
            sca, scb = sc_t[i]
            nc.scalar.activation(
                pt[:, 0:QBLK], sca[:, :],
                mybir.ActivationFunctionType.Exp, scale=0.125,
            )
            if i < 8:
                nc.scalar.activation(
                    pt[:, QBLK:2 * QBLK], scb[:, :],
                    mybir.ActivationFunctionType.Exp, scale=0.125,
                )
            else:
                nc.vector.tensor_scalar(
                    pt[:, QBLK:2 * QBLK].bitcast(I16),
                    scb[:, :], SCH_A16, SCH_B16,
                    op0=mybir.AluOpType.mult, op1=mybir.AluOpType.add,
                )
            pt_t[i] = pt

        def av(i):
            blk, p = divmod(i, NPAIR)
            if p == 0:
                acc[blk] = pso.tile([E1, QBLK], F32, tag="ps_out",
                                    name=f"acc{blk}")
            a = acc[blk]
            nc.tensor.matmul(
                a[:, :], Vp[:, 2 * p, :], pt_t[i][:, 0:QBLK],
                start=(p == 0), stop=False,
            )
            nc.tensor.matmul(
                a[:, :], Vp[:, 2 * p + 1, :], pt_t[i][:, QBLK:2 * QBLK],
                start=False, stop=(p == NPAIR - 1),
            )
            if p == NPAIR - 1:
                # two half-epilogues (bf16) on both HWDGE rings shorten the
                # copy->dma tail after the final pair; the last block runs
                # its two casts on DVE + gpsimd in parallel
                c0 = blk * QBLK
                ob = obp.tile([E1, QBLK], BF16, tag="ob")
                nc.vector.tensor_copy(ob[:, 0:256], a[:, 0:256])
                nc.sync.dma_start(out=out[:, c0:c0 + 256], in_=ob[:, 0:256])
                nc.vector.tensor_copy(ob[:, 256:512], a[:, 256:512])
                # mid-run out-DMAs stay off the scalar engine (its doorbell
                # would steal ~0.7us from the exp stream); the final block
                # splits across both rings to shorten the drain tail
                oeng = nc.scalar if blk == NBLK - 1 else nc.sync
                oeng.dma_start(out=out[:, c0 + 256:c0 + 512],
                               in_=ob[:, 256:512])

        # filler (projection) work attached ahead of specific steps so it
        # lands in the PE's exp-wait slack and tracks DMA arrival order.
        # Constraints: scores(i) consumes k_proj of its KT block at the
        # TOP of step i, av(i-2) consumes Vp tiles before the fillers run,
        # and every tensor's DMA must have landed (deadline-ordered rings).
        fillers = {
            0: [lambda: k_proj_half("ka1", slice(256, 512))],
            1: [lambda: k_proj(1), lambda: v_proj4(0)],
            3: [lambda: k_proj(2), lambda: v_proj4(1)],
            5: [lambda: k_proj(3), lambda: v_proj4(2)],
            7: [lambda: q_proj(1), lambda: v_proj4(3)],
        }
        for b in range(2, NBLK):
            fillers.setdefault(8 * (b - 1) + 4, []).append(
                lambda b=b: q_proj(b))

        # prologue: warmup matmuls bridge the DMA wait so the PE activity
        # window stays continuously busy from ~8.3us until steady state
        warmup(7)
        q_proj(0)
        k_proj_half("ka0", slice(0, 256))

        # skew-2 pipeline: scores run two pairs ahead of the AV consumer so
        # the exp latency (ACT or DVE) never sits on the PE critical path.
        # expq is emitted LAST in each step: the DVE/ACT completion sems
        # are monotonic per engine, so any projection epilogue queued
        # after an exp would drag the exp's ~1.2us into every PE matmul
        # that waits on that epilogue (PSUM-pool reuse).
        scores(0)
        for f in fillers.get(0, []):
            f()
        expq(0)
        scores(1)
        for f in fillers.get(1, []):
            f()
        expq(1)
        for i in range(2, NSTEP):
            scores(i)
            # AV of the pair whose exp completed a step ago goes ahead of
            # the projection fillers, so it is never queued behind ~1us of
            # proj matmuls it does not depend on
            av(i - 2)
            for f in fillers.get(i, []):
                f()
            expq(i)
        av(NSTEP - 2)
        av(NSTEP - 1)


_CACHED_NC = None


def _get_nc():
    global _CACHED_NC
    if _CACHED_NC is None:
        _CACHED_NC = _build_bass()
    return _CACHED_NC


def _swizzle_w(w: np.ndarray, double: bool = False) -> np.ndarray:
    """[512, width] -> [128, NCH*width] with chunk-major free dim.
    double=True emits [W|W] chunks ([128, NCH*2*width]) so one matmul
    writes the projection into both partition halves."""
    width = w.shape[1]
    c = w.reshape(NCH, 128, width)
    if double:
        c = np.concatenate([c, c], axis=2)
        width *= 2
    return np.ascontiguousarray(
        c.transpose(1, 0, 2).reshape(128, NCH * width)
    ).astype(ml_dtypes.bfloat16)


def _chunk3d(xT: np.ndarray, splits, dt) -> dict:
    """[512, N] (transposed input) -> per-split [128, NCH, w] arrays in
    the exact SBUF tile layout (partition p, chunk c) = row c*128+p."""
    x = np.asarray(xT, np.float32).reshape(NCH, 128, xT.shape[1])
    outmaps = {}
    c0 = 0
    for name, w in splits:
        outmaps[name] = np.ascontiguousarray(
            x[:, :, c0:c0 + w].transpose(1, 0, 2)).astype(dt)
        c0 += w
    return outmaps


def _make_in_maps(q, k, v, Wq, bq, Wk, bk, Wv, bv):
    del bk  # constant along the kv axis -> softmax-invariant, dropped
    del bv  # applied on the host after the softmax division
    bf = ml_dtypes.bfloat16
    f8 = ml_dtypes.float8_e3m4
    wq_s = _swizzle_w(np.asarray(Wq, np.float32), double=True)
    wk_s = _swizzle_w(np.asarray(Wk, np.float32), double=True)
    wv_s = _swizzle_w(np.asarray(Wv, np.float32))
    bq_col = np.asarray(bq, np.float32).reshape(E, 1)
    bq_a = np.ascontiguousarray(
        np.concatenate([bq_col, bq_col], axis=0)).astype(bf)  # [128, 1]
    cpk_a = wk_s
    cpq_a = np.ascontiguousarray(np.concatenate([wq_s, bq_a], axis=1))
    cpv_a = wv_s
    assert cpk_a.shape == (128, CPK_W)
    assert cpq_a.shape == (128, CPQ_W)
    assert cpv_a.shape == (128, CPV_W)

    in_maps = []
    for core in range(N_CORES):
        b, h = core // 2, core % 2
        m = {"cpk": cpk_a, "cpq": cpq_a, "cpv": cpv_a}
        m.update(_chunk3d(np.asarray(q[b], np.float32).T, Q_SPLITS, f8))
        m.update(_chunk3d(
            np.asarray(k[b, h * KS:(h + 1) * KS, :], np.float32).T,
            K_SPLITS, f8))
        m.update(_chunk3d(
            np.asarray(v[b, h * KS:(h + 1) * KS, :], np.float32).T,
            V_SPLITS, f8))
        in_maps.append(m)
    return in_maps


def _unshard(results, bv) -> np.ndarray:
    bv_row = np.asarray(bv, np.float32)
    final = np.empty((B, S, E), np.float32)
    for b in range(B):
        o = (np.asarray(results[2 * b]["out"], np.float32)
             + np.asarray(results[2 * b + 1]["out"], np.float32))  # [65, S]
        final[b] = (o[:E] / o[E:E + 1]).T + bv_row
    return final


def kernel(q, k, v, Wq, bq, Wk, bk, Wv, bv, _trace=False):
    nc = _get_nc()
    in_maps = _make_in_maps(q, k, v, Wq, bq, Wk, bk, Wv, bv)
    res = run_bass_kernel_spmd(nc, in_maps, core_ids=list(range(N_CORES)),
                               trace=_trace)
    outp = _unshard(res.results, bv)
    if _trace:
        kernel.last_result = res
    return outp
